# revision 26
# baseline (speedup 1.0000x reference)
"""BiLSTM-CRF Trainium2 kernel: 8-core data-parallel (8 batch items/core).

Strategy:
- Shard batch 64 -> 8 cores x 8 items. Replicate all params.
- Per core: embed gather -> transpose -> L0 input proj (bulk, to DRAM) ->
  L0 fwd+bwd scan (interleaved) -> L1 proj -> L1 scan -> FC -> Viterbi DP
  -> backtrace. All on device.
- Matmuls in bf16 (empirically 0 flipped Viterbi tags vs fp32), fp32
  accumulation in PSUM, fp32 cell state.
- Gate layout transposed: gates on partitions, batch on free dim, so the
  ACT/DVE work runs on 128 lanes. Gate rows permuted to [i,f,o,g] chunk
  order so one sigmoid covers i,f,o contiguously.
- Scan PSUM seeded with the input projection via an identity matmul
  (G = I.T @ pre, then Whh@h accumulates on top) - no separate zeroer
  matmul or DVE pre-add; ACT reads gates straight from PSUM.
- fwd/bwd scans interleave per step so each direction's ACT/DVE tail is
  hidden under the other direction's 64 weight-chunk loads (the PE-bound
  critical path: Whh must stream through the PE every step).
- Embedding table shipped as bf16 (identical numerics; halves upload).
- Recurrent matmul in fp8 e4m3 (both operands: Whh weights + a cast copy of
  h; history h stays bf16). Validated 0/16384 flipped tags in numpy
  emulation. FWL loads fp8 weights 2x faster than bf16, cutting the
  LDW-bound scan cost ~40%. (Mixed fp8 x bf16 operands crash the exec
  unit - both sides must be fp8.)
"""

import os
import sys
from contextlib import ExitStack

import numpy as np
import ml_dtypes

sys.path.insert(0, "/opt/trn_rl_repo")

import concourse.bass as bass
import concourse.tile as tile
from concourse import bacc
from concourse import mybir
from concourse.bass import ds, ts
from concourse.bass_utils import run_bass_kernel_spmd
from concourse.masks import make_identity

F32 = mybir.dt.float32
BF16 = mybir.dt.bfloat16
I32 = mybir.dt.int32
AF = mybir.ActivationFunctionType
OP = mybir.AluOpType
AX = mybir.AxisListType
FP8 = mybir.dt.float8e4

SEQ, BATCH, NCORES = 256, 64, 8
NB = BATCH // NCORES          # 8 batch items per core
NT = SEQ * NB                 # 2048 tokens per core (t-major)
EMB, HID, TAGS = 300, 512, 20
START, STOP = 18, 19
G4 = 4 * HID                  # 2048 gate rows
KH = HID // 128               # 4 hidden k-chunks
MCH = G4 // 128               # 16 gate m-chunks
EMBP = 384                    # padded emb dim (3 x 128)
NBLK = 512                    # token block for projections
SCAN_UNROLL = 64
STATIC_SCAN = False


def _perm_gates():
    """Row permutation: torch order [i,f,g,o] -> chunks [i0..3,f0..3,o0..3,g0..3]."""
    p = []
    for blk in (0, 1, 3, 2):          # i, f, o, g blocks of 512 rows
        for j in range(KH):
            s = blk * HID + j * 128
            p.extend(range(s, s + 128))
    return np.array(p, np.int64)


def build_nc():
    nc = bacc.Bacc("TRN2", target_bir_lowering=False)

    # ---- external inputs (per core) ----
    tok = nc.dram_tensor("tok", [128, NT // 128], I32, kind="ExternalInput")
    emb = nc.dram_tensor("emb", [50000, EMB], BF16, kind="ExternalInput")
    wih = {}
    whh = {}
    bias = {}
    for l in range(2):
        kin = 3 if l == 0 else 8
        for d in ("f", "b"):
            wih[(l, d)] = nc.dram_tensor(f"wih{l}{d}", [128, kin, G4], BF16,
                                         kind="ExternalInput")
            whh[(l, d)] = nc.dram_tensor(f"whh{l}{d}", [128, KH, G4], FP8,
                                         kind="ExternalInput")
            bias[(l, d)] = nc.dram_tensor(f"b{l}{d}", [128, MCH], F32,
                                          kind="ExternalInput")
    fcw = nc.dram_tensor("fcw", [128, 8, TAGS], BF16, kind="ExternalInput")
    trans8 = nc.dram_tensor("trans8", [NB, TAGS * TAGS], F32, kind="ExternalInput")
    trstop = nc.dram_tensor("trstop", [NB, TAGS], F32, kind="ExternalInput")
    v0in = nc.dram_tensor("v0", [NB, TAGS], F32, kind="ExternalInput")
    iota20 = nc.dram_tensor("iota20", [NB, TAGS], F32, kind="ExternalInput")
    iodesc = nc.dram_tensor("iodesc", [NB, TAGS], F32, kind="ExternalInput")

    path = nc.dram_tensor("path", [NB, SEQ], F32, kind="ExternalOutput")

    feats_dram = nc.dram_tensor("featsd", [SEQ, NB, TAGS], F32)
    # internal DRAM for per-step pre-activations  [t][128 part][m*NB+b]
    pre_d = {}
    for l in range(2):
        for d in ("f", "b"):
            pre_d[(l, d)] = nc.dram_tensor(f"pre{l}{d}", [SEQ, 128, MCH * NB], BF16)

    with tile.TileContext(nc) as tc, ExitStack() as ctx:
        pers = ctx.enter_context(tc.tile_pool(name="pers", bufs=1))

        ident = pers.tile([128, 128], BF16)
        make_identity(nc, ident[:])
        zeros_bf = pers.tile([128, 128], BF16)
        nc.vector.memset(zeros_bf[:], 0.0)

        tok_sb = pers.tile([128, NT // 128], I32)
        nc.sync.dma_start(tok_sb[:], tok[:, :])

        # hs layout per (layer, dir): [128 part=hid%128, kchunk, t, b] bf16
        hs = {}
        for l in range(2):
            for d in ("f", "b"):
                hs[(l, d)] = pers.tile([128, KH, SEQ, NB], BF16, name=f"hs{l}{d}", tag=f"hs{l}{d}")

        # ---------- phase 1: gather + transpose x ----------
        with ExitStack() as p1:
            gp = p1.enter_context(tc.tile_pool(name="gather", bufs=3))
            tp = p1.enter_context(tc.tile_pool(name="tpsum", bufs=2, space="PSUM"))
            xT = pers.tile([128, 3, NT], BF16, name="xT")
            for c in range(NT // 128):
                xb = gp.tile([128, EMBP], BF16, tag="xb")
                nc.vector.memset(xb[:, EMB:EMBP], 0.0)
                nc.gpsimd.indirect_dma_start(
                    out=xb[:, 0:EMB], out_offset=None,
                    in_=emb[:, :],
                    in_offset=bass.IndirectOffsetOnAxis(ap=tok_sb[:, c:c + 1], axis=0),
                )
                for k in range(3):
                    pt = tp.tile([128, 128], BF16, tag="tp")
                    nc.tensor.transpose(pt[:], xb[:, ts(k, 128)], ident[:])
                    nc.vector.tensor_copy(xT[:, k, ts(c, 128)], pt[:])

        # ---------- helper: bulk input projection for one (layer, dir) ----------
        def proj(l, d, wih_sb, rhs_fn, pool, ppool, spool):
            bias_sb = pool.tile([128, MCH], F32, tag="bias")
            nc.sync.dma_start(bias_sb[:], bias[(l, d)][:, :])
            kin = 3 if l == 0 else 8
            for nb in range(NT // NBLK):
                stage = spool.tile([128, NBLK // NB, MCH, NB], BF16, tag="stage")
                for m in range(MCH):
                    pt = ppool.tile([128, NBLK], F32, tag="pp")
                    for k in range(kin):
                        nc.tensor.matmul(
                            pt[:], wih_sb[:, k, ts(m, 128)], rhs_fn(k, nb),
                            start=(k == 0), stop=(k == kin - 1),
                        )
                    nc.vector.tensor_tensor(
                        out=stage[:, :, m, :],
                        in0=pt[:].rearrange("p (t b) -> p t b", b=NB),
                        in1=bias_sb[:, m:m + 1, None].to_broadcast(
                            (128, NBLK // NB, NB)),
                        op=OP.add,
                    )
                nc.sync.dma_start(
                    pre_d[(l, d)][ds(nb * (NBLK // NB), NBLK // NB), :, :]
                    .rearrange("t p c -> p t c"),
                    stage[:].rearrange("p t m b -> p t (m b)"),
                )

        # ---------- helper: scan for one layer ----------
        def scan(l, spp, sp, gp_psum):
            c_sb = {d: spp.tile([128, KH * NB], F32, tag=f"c{d}", name=f"c{d}") for d in ("f", "b")}
            h_cur = {d: spp.tile([128, KH * NB], BF16, tag=f"h{d}", name=f"h{d}") for d in ("f", "b")}
            h_q = {d: spp.tile([128, KH * NB], FP8, tag=f"hq{d}", name=f"hq{d}") for d in ("f", "b")}
            whh_sb = {}
            for d in ("f", "b"):
                whh_sb[d] = spp.tile([128, KH, G4], FP8, tag=f"whh{d}", name=f"whh{d}")
                nc.sync.dma_start(whh_sb[d][:], whh[(l, d)][:, :, :])

            def step(d, t_new, t_prev, first):
                pre_t = sp.tile([128, MCH * NB], BF16, tag=f"pre{d}", name=f"pre{d}")
                nc.sync.dma_start(
                    pre_t[:].rearrange("p (o c) -> p o c", o=1),
                    pre_d[(l, d)].rearrange("t p c -> p t c")[:, ds(t_new, 1), :],
                )
                if first:
                    gin = pre_t
                else:
                    G = gp_psum.tile([128, MCH * NB], F32, tag=f"G{d}", name=f"G{d}")
                    # G = I.T @ pre_t (seeds PSUM with pre and zeroes the bank),
                    # then accumulate Whh @ h on top.
                    nc.tensor.matmul(G[:], ident[:], pre_t[:],
                                     start=True, stop=False, skip_group_check=True)
                    for k in range(KH):
                        rhs = h_q[d][:, ts(k, NB)]
                        for m in range(MCH):
                            nc.tensor.matmul(
                                G[:, ts(m, NB)], whh_sb[d][:, k, ts(m, 128)], rhs,
                                start=False, stop=(k == KH - 1 and m == MCH - 1),
                                skip_group_check=True,
                            )
                    gin = G
                ns = KH * NB  # 32 cols per gate group
                sg = sp.tile([128, 3 * ns], F32, tag=f"sg{d}", name=f"sg{d}")
                nc.scalar.activation(sg[:], gin[:, 0:3 * ns], AF.Sigmoid)
                tg = sp.tile([128, ns], F32, tag=f"tg{d}", name=f"tg{d}")
                nc.scalar.activation(tg[:], gin[:, 3 * ns:4 * ns], AF.Tanh)
                t2 = sp.tile([128, ns], F32, tag=f"t2{d}", name=f"t2{d}")
                nc.vector.tensor_mul(t2[:], sg[:, 0:ns], tg[:])
                if first:
                    nc.vector.tensor_copy(c_sb[d][:], t2[:])
                else:
                    t1 = sp.tile([128, ns], F32, tag=f"t1{d}", name=f"t1{d}")
                    nc.vector.tensor_mul(t1[:], sg[:, ns:2 * ns], c_sb[d][:])
                    nc.vector.tensor_add(c_sb[d][:], t1[:], t2[:])
                th = sp.tile([128, ns], F32, tag=f"th{d}", name=f"th{d}")
                nc.scalar.activation(th[:], c_sb[d][:], AF.Tanh)
                nc.vector.tensor_mul(out=h_cur[d][:], in0=sg[:, 2 * ns:3 * ns],
                                     in1=th[:])
                nc.vector.tensor_copy(h_q[d][:], h_cur[d][:])
                nc.sync.dma_start(
                    hs[(l, d)][:, :, ds(t_new, 1), :],
                    h_cur[d][:].rearrange("p (k b) -> p k b", b=NB)[:, :, None, :],
                )

            step("f", 0, None, True)
            step("b", SEQ - 1, None, True)

            def body(iv):
                step("f", iv, iv - 1, False)
                step("b", 255 - iv, 256 - iv, False)

            if STATIC_SCAN:
                for i in range(1, SEQ):
                    body(i)
            else:
                tc.For_i_unrolled(1, SEQ, 1, body, max_unroll=SCAN_UNROLL)

        # ---------- layer 0 ----------
        with ExitStack() as p2:
            pool = p2.enter_context(tc.tile_pool(name="l0p", bufs=1))
            ppool = p2.enter_context(tc.tile_pool(name="l0psum", bufs=2, space="PSUM"))
            spool = p2.enter_context(tc.tile_pool(name="l0stage", bufs=2))
            wih_sb = {}
            for d in ("f", "b"):
                wih_sb[d] = pool.tile([128, 3, G4], BF16, tag=f"wih{d}", name=f"wih{d}")
                nc.sync.dma_start(wih_sb[d][:], wih[(0, d)][:, :, :])
            for d in ("f", "b"):
                proj(0, d, wih_sb[d],
                     lambda k, nb: xT[:, k, ts(nb, NBLK)],
                     pool, ppool, spool)

        with ExitStack() as p3:
            spp = p3.enter_context(tc.tile_pool(name="scan0p", bufs=1))
            sp = p3.enter_context(tc.tile_pool(name="scan0", bufs=4))
            gp_psum = p3.enter_context(tc.tile_pool(name="G0", bufs=2, space="PSUM"))
            scan(0, spp, sp, gp_psum)

        # ---------- layer 1 ----------
        def l1rhs(k, nb):
            src = hs[(0, "f")] if k < KH else hs[(0, "b")]
            return src[:, k % KH, ts(nb, NBLK // NB), :].rearrange("p t b -> p (t b)")

        with ExitStack() as p4:
            pool = p4.enter_context(tc.tile_pool(name="l1p", bufs=1))
            ppool = p4.enter_context(tc.tile_pool(name="l1psum", bufs=2, space="PSUM"))
            spool = p4.enter_context(tc.tile_pool(name="l1stage", bufs=2))
            wih_sb = {}
            for d in ("f", "b"):
                wih_sb[d] = pool.tile([128, 8, G4], BF16, tag=f"wih1{d}", name=f"wih1{d}")
                nc.sync.dma_start(wih_sb[d][:], wih[(1, d)][:, :, :])
            for d in ("f", "b"):
                proj(1, d, wih_sb[d], l1rhs, pool, ppool, spool)

        with ExitStack() as p5:
            spp = p5.enter_context(tc.tile_pool(name="scan1p", bufs=1))
            sp = p5.enter_context(tc.tile_pool(name="scan1", bufs=4))
            gp_psum = p5.enter_context(tc.tile_pool(name="G1", bufs=2, space="PSUM"))
            scan(1, spp, sp, gp_psum)

        # ---------- FC -> feats ----------
        with ExitStack() as p6:
            fp = p6.enter_context(tc.tile_pool(name="fc", bufs=2))
            fps = p6.enter_context(tc.tile_pool(name="fcpsum", bufs=4, space="PSUM"))
            fcw_sb = pers.tile([128, 8, TAGS], BF16)
            nc.sync.dma_start(fcw_sb[:], fcw[:, :, :])
            for c in range(NT // 128):
                pt = fps.tile([128, TAGS], F32, tag="fcp")
                for k in range(8):
                    src = hs[(1, "f")] if k < KH else hs[(1, "b")]
                    lhsT = src[:, k % KH, ts(c, 128 // NB), :].rearrange(
                        "p t b -> p (t b)")
                    nc.tensor.matmul(pt[:], lhsT, fcw_sb[:, k, :],
                                     start=(k == 0), stop=(k == 7))
                fstage = fp.tile([128, TAGS], F32, tag="fstage")
                nc.vector.tensor_copy(fstage[:], pt[:])
                nc.sync.dma_start(
                    feats_dram[ts(c, 128 // NB), :, :].rearrange(
                        "t b q -> (t b) q"),
                    fstage[:],
                )
        featsV = pers.tile([NB, SEQ * TAGS], F32, name="featsV")
        nc.sync.dma_start(featsV[:].rearrange("b (t q) -> b t q", q=TAGS),
                          feats_dram[:].rearrange("t b q -> b t q"))

        # ---------- Viterbi ----------
        vp = ctx.enter_context(tc.tile_pool(name="vit", bufs=2))
        vpers = ctx.enter_context(tc.tile_pool(name="vitp", bufs=1))
        tr_sb = vpers.tile([NB, TAGS * TAGS], F32)
        nc.sync.dma_start(tr_sb[:], trans8[:, :])
        trs_sb = vpers.tile([NB, TAGS], F32)
        nc.sync.dma_start(trs_sb[:], trstop[:, :])
        io20 = vpers.tile([NB, TAGS], F32)
        nc.sync.dma_start(io20[:], iota20[:, :])
        iod = vpers.tile([NB, TAGS], F32)
        nc.sync.dma_start(iod[:], iodesc[:, :])
        v = vpers.tile([NB, TAGS], F32, name="vstate")
        nc.sync.dma_start(v[:], v0in[:, :])
        bp_sb = vpers.tile([NB, SEQ * TAGS], F32, name="bps")
        tags_sb = vpers.tile([NB, SEQ], F32, name="tags")

        for t in range(SEQ):
            s = vp.tile([NB, TAGS * TAGS], F32, tag="s")
            nc.vector.tensor_tensor(
                out=s[:].rearrange("p (n q) -> p n q", q=TAGS),
                in0=v[:, None, :].to_broadcast((NB, TAGS, TAGS)),
                in1=tr_sb[:].rearrange("p (n q) -> p n q", q=TAGS),
                op=OP.add,
            )
            vmax = vp.tile([NB, TAGS], F32, tag="vmax")
            nc.vector.reduce_max(vmax[:], s[:].rearrange("p (n q) -> p n q", q=TAGS),
                                 axis=AX.X)
            eq = vp.tile([NB, TAGS * TAGS], F32, tag="eq")
            nc.vector.tensor_tensor(
                out=eq[:].rearrange("p (n q) -> p n q", q=TAGS),
                in0=s[:].rearrange("p (n q) -> p n q", q=TAGS),
                in1=vmax[:, :, None].to_broadcast((NB, TAGS, TAGS)),
                op=OP.is_equal,
            )
            prod = vp.tile([NB, TAGS * TAGS], F32, tag="prod")
            nc.gpsimd.tensor_tensor(
                out=prod[:].rearrange("p (n q) -> p n q", q=TAGS),
                in0=eq[:].rearrange("p (n q) -> p n q", q=TAGS),
                in1=iod[:, None, :].to_broadcast((NB, TAGS, TAGS)),
                op=OP.mult,
            )
            nc.vector.reduce_max(
                bp_sb[:, ts(t, TAGS)],
                prod[:].rearrange("p (n q) -> p n q", q=TAGS), axis=AX.X)
            nc.vector.tensor_add(v[:], vmax[:], featsV[:, ts(t, TAGS)])

        # last tag = argmax(v + trans[STOP])
        s2 = vp.tile([NB, TAGS], F32, tag="s")
        nc.vector.tensor_add(s2[:], v[:], trs_sb[:])
        m2 = vp.tile([NB, 1], F32, tag="m2")
        nc.vector.reduce_max(m2[:], s2[:], axis=AX.X)
        eq2 = vp.tile([NB, TAGS], F32, tag="eq")
        nc.vector.tensor_tensor(out=eq2[:], in0=s2[:],
                                in1=m2[:].to_broadcast((NB, TAGS)), op=OP.is_equal)
        pr2 = vp.tile([NB, TAGS], F32, tag="prod")
        nc.vector.tensor_mul(pr2[:], eq2[:], iod[:])
        enc = vp.tile([NB, 1], F32, tag="enc")
        nc.vector.reduce_max(enc[:], pr2[:], axis=AX.X)
        tag = vpers.tile([NB, 1], F32, name="tag")
        nc.vector.tensor_scalar(out=tag[:], in0=enc[:], scalar1=-1.0,
                                scalar2=float(TAGS), op0=OP.mult, op1=OP.add)

        for t in range(SEQ - 1, -1, -1):
            nc.vector.tensor_copy(tags_sb[:, t:t + 1], tag[:])
            if t == 0:
                break
            pr = vp.tile([NB, TAGS], F32, tag="prod")
            nc.vector.scalar_tensor_tensor(
                out=pr[:], in0=io20[:], scalar=tag[:], in1=bp_sb[:, ts(t, TAGS)],
                op0=OP.is_equal, op1=OP.mult)
            enc2 = vp.tile([NB, 1], F32, tag="enc")
            nc.vector.reduce_max(enc2[:], pr[:], axis=AX.X)
            nc.vector.tensor_scalar(out=tag[:], in0=enc2[:], scalar1=-1.0,
                                    scalar2=float(TAGS), op0=OP.mult, op1=OP.add)

        nc.sync.dma_start(path[:, :], tags_sb[:])

    nc.compile()
    return nc


_NC_CACHE = {}


def _get_nc():
    if "nc" not in _NC_CACHE:
        _NC_CACHE["nc"] = build_nc()
    return _NC_CACHE["nc"]


def kernel(sentence, emb, lstm_params, fc_W, fc_b, transitions):
    sentence = np.asarray(sentence)
    emb = np.asarray(emb, np.float32)
    fc_W = np.asarray(fc_W, np.float32)
    fc_b = np.asarray(fc_b, np.float32)
    transitions = np.asarray(transitions, np.float32)
    perm = _perm_gates()

    common = {"emb": emb.astype(ml_dtypes.bfloat16)}
    for l, lp in enumerate(lstm_params):
        kin = EMB if l == 0 else 2 * HID
        kpad = EMBP if l == 0 else 2 * HID
        for di, d in enumerate(("f", "b")):
            Wih = np.asarray(lp[3 * di + 0], np.float32)[perm]      # [2048, kin]
            Whh = np.asarray(lp[3 * di + 1], np.float32)[perm]      # [2048, 512]
            b = np.asarray(lp[3 * di + 2], np.float32)[perm]        # [2048]
            WihT = np.zeros((kpad, G4), np.float32)
            WihT[:kin] = Wih.T
            common[f"wih{l}{d}"] = np.ascontiguousarray(
                WihT.reshape(kpad // 128, 128, G4).transpose(1, 0, 2)
            ).astype(ml_dtypes.bfloat16)
            common[f"whh{l}{d}"] = np.ascontiguousarray(
                Whh.T.reshape(KH, 128, G4).transpose(1, 0, 2)
            ).astype(ml_dtypes.float8_e4m3fn)
            common[f"b{l}{d}"] = np.ascontiguousarray(
                b.reshape(MCH, 128).T).astype(np.float32)
    common["fcw"] = np.ascontiguousarray(
        fc_W.T.reshape(8, 128, TAGS).transpose(1, 0, 2)).astype(ml_dtypes.bfloat16)

    transp = transitions + fc_b[:, None]   # fold fc bias into transition rows
    common["trans8"] = np.tile(transp.reshape(1, -1), (NB, 1)).astype(np.float32)
    common["trstop"] = np.tile(transitions[STOP].reshape(1, -1), (NB, 1)).astype(
        np.float32)
    v0 = np.full((TAGS,), -10000.0, np.float32)
    v0[START] = 0.0
    common["v0"] = np.tile(v0.reshape(1, -1), (NB, 1))
    common["iota20"] = np.tile(np.arange(TAGS, dtype=np.float32).reshape(1, -1),
                               (NB, 1))
    common["iodesc"] = np.tile(
        (TAGS - np.arange(TAGS, dtype=np.float32)).reshape(1, -1), (NB, 1))

    in_maps = []
    for i in range(NCORES):
        m = dict(common)
        toks = np.asarray(sentence[:, i * NB:(i + 1) * NB], np.int32).reshape(-1)
        m["tok"] = np.ascontiguousarray(toks.reshape(NT // 128, 128).T)
        in_maps.append(m)

    nc = _get_nc()
    trace = os.environ.get("KERNEL_TRACE", "0") == "1"
    res = run_bass_kernel_spmd(nc, in_maps, core_ids=list(range(NCORES)),
                               trace=trace)
    if trace:
        print(f"HW exec time: {res.exec_time_ns} ns")
    out = np.concatenate([r["path"] for r in res.results], axis=0)
    return out.astype(np.float32)


if __name__ == "__main__":
    sys.path.insert(0, "/root/problem")
    from reference import setup_inputs, reference
    inp = setup_inputs()
    inp = {k: np.asarray(v) if not isinstance(v, tuple) else v
           for k, v in inp.items()}
    got = kernel(**inp)
    print("kernel output", got.shape, got.dtype)


# revision 27
# speedup vs baseline: 1.0768x; 1.0768x over previous
"""BiLSTM-CRF Trainium2 kernel: 8-core data-parallel (8 batch items/core).

Strategy:
- Shard batch 64 -> 8 cores x 8 items. Replicate all params.
- Per core: embed gather -> transpose -> L0 input proj (bulk, to DRAM) ->
  L0 fwd+bwd scan (interleaved) -> L1 proj -> L1 scan -> FC -> Viterbi DP
  -> backtrace. All on device.
- Matmuls in bf16 (empirically 0 flipped Viterbi tags vs fp32), fp32
  accumulation in PSUM, fp32 cell state.
- Gate layout transposed: gates on partitions, batch on free dim, so the
  ACT/DVE work runs on 128 lanes. Gate rows permuted to [i,f,o,g] chunk
  order so one sigmoid covers i,f,o contiguously.
- Scan PSUM seeded with the input projection via an identity matmul
  (G = I.T @ pre, then Whh@h accumulates on top) - no separate zeroer
  matmul or DVE pre-add; ACT reads gates straight from PSUM.
- fwd/bwd scans interleave per step so each direction's ACT/DVE tail is
  hidden under the other direction's 64 weight-chunk loads (the PE-bound
  critical path: Whh must stream through the PE every step).
- Embedding table shipped as bf16 (identical numerics; halves upload).
- Recurrent matmul in fp8 e4m3 (both operands: Whh weights + a cast copy of
  h; history h stays bf16). Validated 0/16384 flipped tags in numpy
  emulation. FWL loads fp8 weights 2x faster than bf16, cutting the
  LDW-bound scan cost ~40%. (Mixed fp8 x bf16 operands crash the exec
  unit - both sides must be fp8.)
"""

import os
import sys
from contextlib import ExitStack

import numpy as np
import ml_dtypes

sys.path.insert(0, "/opt/trn_rl_repo")

import concourse.bass as bass
import concourse.tile as tile
from concourse import bacc
from concourse import mybir
from concourse.bass import ds, ts
from concourse.bass_utils import run_bass_kernel_spmd
from concourse.masks import make_identity

F32 = mybir.dt.float32
BF16 = mybir.dt.bfloat16
I32 = mybir.dt.int32
AF = mybir.ActivationFunctionType
OP = mybir.AluOpType
AX = mybir.AxisListType
FP8 = mybir.dt.float8e4

SEQ, BATCH, NCORES = 256, 64, 8
NB = BATCH // NCORES          # 8 batch items per core
NT = SEQ * NB                 # 2048 tokens per core (t-major)
EMB, HID, TAGS = 300, 512, 20
START, STOP = 18, 19
G4 = 4 * HID                  # 2048 gate rows
KH = HID // 128               # 4 hidden k-chunks
MCH = G4 // 128               # 16 gate m-chunks
EMBP = 384                    # padded emb dim (3 x 128)
NBLK = 512                    # token block for projections
SCAN_UNROLL = 64
STATIC_SCAN = False


def _perm_gates():
    """Row permutation: torch order [i,f,g,o] -> chunks [i0..3,f0..3,o0..3,g0..3]."""
    p = []
    for blk in (0, 1, 3, 2):          # i, f, o, g blocks of 512 rows
        for j in range(KH):
            s = blk * HID + j * 128
            p.extend(range(s, s + 128))
    return np.array(p, np.int64)


def build_nc():
    nc = bacc.Bacc("TRN2", target_bir_lowering=False)

    # ---- external inputs (per core) ----
    tok = nc.dram_tensor("tok", [128, NT // 128], I32, kind="ExternalInput")
    emb = nc.dram_tensor("emb", [50000, EMB], BF16, kind="ExternalInput")
    wih = {}
    whh = {}
    bias = {}
    for l in range(2):
        kin = 3 if l == 0 else 8
        for d in ("f", "b"):
            wih[(l, d)] = nc.dram_tensor(f"wih{l}{d}", [128, kin, G4], BF16,
                                         kind="ExternalInput")
            whh[(l, d)] = nc.dram_tensor(f"whh{l}{d}", [128, KH, G4], FP8,
                                         kind="ExternalInput")
            bias[(l, d)] = nc.dram_tensor(f"b{l}{d}", [128, MCH], F32,
                                          kind="ExternalInput")
    fcw = nc.dram_tensor("fcw", [128, 8, TAGS], BF16, kind="ExternalInput")
    trans8 = nc.dram_tensor("trans8", [NB, TAGS * TAGS], F32, kind="ExternalInput")
    trstop = nc.dram_tensor("trstop", [NB, TAGS], F32, kind="ExternalInput")
    v0in = nc.dram_tensor("v0", [NB, TAGS], F32, kind="ExternalInput")
    iota20 = nc.dram_tensor("iota20", [NB, TAGS], F32, kind="ExternalInput")
    iodesc = nc.dram_tensor("iodesc", [NB, TAGS], F32, kind="ExternalInput")

    path = nc.dram_tensor("path", [NB, SEQ], F32, kind="ExternalOutput")

    feats_dram = nc.dram_tensor("featsd", [SEQ, NB, TAGS], F32)
    # internal DRAM for per-step pre-activations  [t][128 part][m*NB+b]
    pre_d = {}
    for l in range(2):
        for d in ("f", "b"):
            pre_d[(l, d)] = nc.dram_tensor(f"pre{l}{d}", [SEQ, 128, MCH * NB], BF16)

    with tile.TileContext(nc) as tc, ExitStack() as ctx:
        pers = ctx.enter_context(tc.tile_pool(name="pers", bufs=1))

        ident = pers.tile([128, 128], BF16)
        make_identity(nc, ident[:])
        zeros_bf = pers.tile([128, 128], BF16)
        nc.vector.memset(zeros_bf[:], 0.0)

        tok_sb = pers.tile([128, NT // 128], I32)
        nc.sync.dma_start(tok_sb[:], tok[:, :])

        # hs layout per (layer, dir): [128 part=hid%128, kchunk, t, b] bf16
        hs = {}
        for l in range(2):
            for d in ("f", "b"):
                hs[(l, d)] = pers.tile([128, KH, SEQ, NB], BF16, name=f"hs{l}{d}", tag=f"hs{l}{d}")

        # ---------- phase 1: gather + transpose x ----------
        with ExitStack() as p1:
            gp = p1.enter_context(tc.tile_pool(name="gather", bufs=3))
            tp = p1.enter_context(tc.tile_pool(name="tpsum", bufs=2, space="PSUM"))
            xT = pers.tile([128, 3, NT], BF16, name="xT")
            for c in range(NT // 128):
                xb = gp.tile([128, EMBP], BF16, tag="xb")
                nc.vector.memset(xb[:, EMB:EMBP], 0.0)
                nc.gpsimd.indirect_dma_start(
                    out=xb[:, 0:EMB], out_offset=None,
                    in_=emb[:, :],
                    in_offset=bass.IndirectOffsetOnAxis(ap=tok_sb[:, c:c + 1], axis=0),
                )
                for k in range(3):
                    pt = tp.tile([128, 128], BF16, tag="tp")
                    nc.tensor.transpose(pt[:], xb[:, ts(k, 128)], ident[:])
                    nc.vector.tensor_copy(xT[:, k, ts(c, 128)], pt[:])

        # ---------- helper: bulk input projection for one (layer, dir) ----------
        def proj(l, d, wih_sb, rhs_fn, pool, ppool, spool):
            bias_sb = pool.tile([128, MCH], F32, tag="bias")
            nc.sync.dma_start(bias_sb[:], bias[(l, d)][:, :])
            kin = 3 if l == 0 else 8
            for nb in range(NT // NBLK):
                stage = spool.tile([128, NBLK // NB, MCH, NB], BF16, tag="stage")
                for m in range(MCH):
                    pt = ppool.tile([128, NBLK], F32, tag="pp")
                    for k in range(kin):
                        nc.tensor.matmul(
                            pt[:], wih_sb[:, k, ts(m, 128)], rhs_fn(k, nb),
                            start=(k == 0), stop=(k == kin - 1),
                        )
                    nc.vector.tensor_tensor(
                        out=stage[:, :, m, :],
                        in0=pt[:].rearrange("p (t b) -> p t b", b=NB),
                        in1=bias_sb[:, m:m + 1, None].to_broadcast(
                            (128, NBLK // NB, NB)),
                        op=OP.add,
                    )
                nc.sync.dma_start(
                    pre_d[(l, d)][ds(nb * (NBLK // NB), NBLK // NB), :, :]
                    .rearrange("t p c -> p t c"),
                    stage[:].rearrange("p t m b -> p t (m b)"),
                )

        # ---------- helper: scan for one layer ----------
        def scan(l, spp, sp, gp_psum):
            c_sb = {d: spp.tile([128, KH * NB], F32, tag=f"c{d}", name=f"c{d}") for d in ("f", "b")}
            h_cur = {d: spp.tile([128, KH * NB], BF16, tag=f"h{d}", name=f"h{d}") for d in ("f", "b")}
            h_q = {d: spp.tile([128, KH * NB], FP8, tag=f"hq{d}", name=f"hq{d}") for d in ("f", "b")}
            whh_sb = {}
            for d in ("f", "b"):
                whh_sb[d] = spp.tile([128, KH, G4], FP8, tag=f"whh{d}", name=f"whh{d}")
                nc.sync.dma_start(whh_sb[d][:], whh[(l, d)][:, :, :])

            def step(d, t_new, t_prev, first):
                pre_t = sp.tile([128, MCH * NB], BF16, tag=f"pre{d}", name=f"pre{d}")
                nc.sync.dma_start(
                    pre_t[:].rearrange("p (o c) -> p o c", o=1),
                    pre_d[(l, d)].rearrange("t p c -> p t c")[:, ds(t_new, 1), :],
                )
                if first:
                    gin = pre_t
                else:
                    G = gp_psum.tile([128, MCH * NB], F32, tag=f"G{d}", name=f"G{d}")
                    # G = I.T @ pre_t (seeds PSUM with pre and zeroes the bank),
                    # then accumulate Whh @ h on top.
                    nc.tensor.matmul(G[:], ident[:], pre_t[:],
                                     start=True, stop=False, skip_group_check=True)
                    for k in range(KH):
                        rhs = h_q[d][:, ts(k, NB)]
                        for m in range(MCH):
                            nc.tensor.matmul(
                                G[:, ts(m, NB)], whh_sb[d][:, k, ts(m, 128)], rhs,
                                start=False, stop=(k == KH - 1 and m == MCH - 1),
                                skip_group_check=True,
                            )
                    gin = G
                ns = KH * NB  # 32 cols per gate group
                sg = sp.tile([128, 3 * ns], F32, tag=f"sg{d}", name=f"sg{d}")
                nc.scalar.activation(sg[:], gin[:, 0:3 * ns], AF.Sigmoid)
                tg = sp.tile([128, ns], F32, tag=f"tg{d}", name=f"tg{d}")
                nc.scalar.activation(tg[:], gin[:, 3 * ns:4 * ns], AF.Tanh)
                t2 = sp.tile([128, ns], F32, tag=f"t2{d}", name=f"t2{d}")
                nc.vector.tensor_mul(t2[:], sg[:, 0:ns], tg[:])
                if first:
                    nc.vector.tensor_copy(c_sb[d][:], t2[:])
                else:
                    t1 = sp.tile([128, ns], F32, tag=f"t1{d}", name=f"t1{d}")
                    nc.vector.tensor_mul(t1[:], sg[:, ns:2 * ns], c_sb[d][:])
                    nc.vector.tensor_add(c_sb[d][:], t1[:], t2[:])
                th = sp.tile([128, ns], F32, tag=f"th{d}", name=f"th{d}")
                nc.scalar.activation(th[:], c_sb[d][:], AF.Tanh)
                nc.vector.tensor_mul(out=h_cur[d][:], in0=sg[:, 2 * ns:3 * ns],
                                     in1=th[:])
                nc.vector.tensor_copy(h_q[d][:], h_cur[d][:])
                nc.sync.dma_start(
                    hs[(l, d)][:, :, ds(t_new, 1), :],
                    h_cur[d][:].rearrange("p (k b) -> p k b", b=NB)[:, :, None, :],
                )

            step("f", 0, None, True)
            step("b", SEQ - 1, None, True)

            def body(iv):
                step("f", iv, iv - 1, False)
                step("b", 255 - iv, 256 - iv, False)

            if STATIC_SCAN:
                for i in range(1, SEQ):
                    body(i)
            else:
                tc.For_i_unrolled(1, SEQ, 1, body, max_unroll=SCAN_UNROLL)

        # ---------- layer 0 ----------
        with ExitStack() as p2:
            pool = p2.enter_context(tc.tile_pool(name="l0p", bufs=1))
            ppool = p2.enter_context(tc.tile_pool(name="l0psum", bufs=2, space="PSUM"))
            spool = p2.enter_context(tc.tile_pool(name="l0stage", bufs=2))
            wih_sb = {}
            for d in ("f", "b"):
                wih_sb[d] = pool.tile([128, 3, G4], BF16, tag=f"wih{d}", name=f"wih{d}")
                nc.sync.dma_start(wih_sb[d][:], wih[(0, d)][:, :, :])
            for d in ("f", "b"):
                proj(0, d, wih_sb[d],
                     lambda k, nb: xT[:, k, ts(nb, NBLK)],
                     pool, ppool, spool)

        with ExitStack() as p3:
            spp = p3.enter_context(tc.tile_pool(name="scan0p", bufs=1))
            sp = p3.enter_context(tc.tile_pool(name="scan0", bufs=4))
            gp_psum = p3.enter_context(tc.tile_pool(name="G0", bufs=2, space="PSUM"))
            scan(0, spp, sp, gp_psum)

        # ---------- layer 1 ----------
        def l1rhs(k, nb):
            src = hs[(0, "f")] if k < KH else hs[(0, "b")]
            return src[:, k % KH, ts(nb, NBLK // NB), :].rearrange("p t b -> p (t b)")

        with ExitStack() as p4:
            pool = p4.enter_context(tc.tile_pool(name="l1p", bufs=1))
            ppool = p4.enter_context(tc.tile_pool(name="l1psum", bufs=2, space="PSUM"))
            spool = p4.enter_context(tc.tile_pool(name="l1stage", bufs=2))
            wih_sb = {}
            for d in ("f", "b"):
                wih_sb[d] = pool.tile([128, 8, G4], BF16, tag=f"wih1{d}", name=f"wih1{d}")
                nc.sync.dma_start(wih_sb[d][:], wih[(1, d)][:, :, :])
            for d in ("f", "b"):
                proj(1, d, wih_sb[d], l1rhs, pool, ppool, spool)

        with ExitStack() as p5:
            spp = p5.enter_context(tc.tile_pool(name="scan1p", bufs=1))
            sp = p5.enter_context(tc.tile_pool(name="scan1", bufs=4))
            gp_psum = p5.enter_context(tc.tile_pool(name="G1", bufs=2, space="PSUM"))
            scan(1, spp, sp, gp_psum)

        # ---------- FC -> feats ----------
        with ExitStack() as p6:
            fp = p6.enter_context(tc.tile_pool(name="fc", bufs=2))
            fps = p6.enter_context(tc.tile_pool(name="fcpsum", bufs=4, space="PSUM"))
            fcw_sb = pers.tile([128, 8, TAGS], BF16)
            nc.sync.dma_start(fcw_sb[:], fcw[:, :, :])
            for c in range(NT // 128):
                pt = fps.tile([128, TAGS], F32, tag="fcp")
                for k in range(8):
                    src = hs[(1, "f")] if k < KH else hs[(1, "b")]
                    lhsT = src[:, k % KH, ts(c, 128 // NB), :].rearrange(
                        "p t b -> p (t b)")
                    nc.tensor.matmul(pt[:], lhsT, fcw_sb[:, k, :],
                                     start=(k == 0), stop=(k == 7))
                fstage = fp.tile([128, TAGS], F32, tag="fstage")
                nc.vector.tensor_copy(fstage[:], pt[:])
                nc.sync.dma_start(
                    feats_dram[ts(c, 128 // NB), :, :].rearrange(
                        "t b q -> (t b) q"),
                    fstage[:],
                )
        featsV = pers.tile([NB, SEQ * TAGS], F32, name="featsV")
        nc.sync.dma_start(featsV[:].rearrange("b (t q) -> b t q", q=TAGS),
                          feats_dram[:].rearrange("t b q -> b t q"))

        # ---------- Viterbi ----------
        vp = ctx.enter_context(tc.tile_pool(name="vit", bufs=2))
        vpers = ctx.enter_context(tc.tile_pool(name="vitp", bufs=1))
        tr_sb = vpers.tile([NB, TAGS * TAGS], F32)
        nc.sync.dma_start(tr_sb[:], trans8[:, :])
        trs_sb = vpers.tile([NB, TAGS], F32)
        nc.sync.dma_start(trs_sb[:], trstop[:, :])
        io20 = vpers.tile([NB, TAGS], F32)
        nc.sync.dma_start(io20[:], iota20[:, :])
        iod = vpers.tile([NB, TAGS], F32)
        nc.sync.dma_start(iod[:], iodesc[:, :])
        v = vpers.tile([NB, TAGS], F32, name="vstate")
        nc.sync.dma_start(v[:], v0in[:, :])
        bp_sb = vpers.tile([NB, SEQ * TAGS], F32, name="bps")
        tags_sb = vpers.tile([NB, SEQ], F32, name="tags")

        for t in range(SEQ):
            s = vp.tile([NB, TAGS * TAGS], F32, tag="s")
            nc.vector.tensor_tensor(
                out=s[:].rearrange("p (n q) -> p n q", q=TAGS),
                in0=v[:, None, :].to_broadcast((NB, TAGS, TAGS)),
                in1=tr_sb[:].rearrange("p (n q) -> p n q", q=TAGS),
                op=OP.add,
            )
            vmax = vp.tile([NB, TAGS], F32, tag="vmax")
            nc.vector.reduce_max(vmax[:], s[:].rearrange("p (n q) -> p n q", q=TAGS),
                                 axis=AX.X)
            eq = vp.tile([NB, TAGS * TAGS], F32, tag="eq")
            nc.vector.tensor_tensor(
                out=eq[:].rearrange("p (n q) -> p n q", q=TAGS),
                in0=s[:].rearrange("p (n q) -> p n q", q=TAGS),
                in1=vmax[:, :, None].to_broadcast((NB, TAGS, TAGS)),
                op=OP.is_equal,
            )
            prod = vp.tile([NB, TAGS * TAGS], F32, tag="prod")
            nc.gpsimd.tensor_tensor(
                out=prod[:].rearrange("p (n q) -> p n q", q=TAGS),
                in0=eq[:].rearrange("p (n q) -> p n q", q=TAGS),
                in1=iod[:, None, :].to_broadcast((NB, TAGS, TAGS)),
                op=OP.mult,
            )
            nc.vector.reduce_max(
                bp_sb[:, ts(t, TAGS)],
                prod[:].rearrange("p (n q) -> p n q", q=TAGS), axis=AX.X)
            nc.vector.tensor_add(v[:], vmax[:], featsV[:, ts(t, TAGS)])

        # last tag = argmax(v + trans[STOP])
        s2 = vp.tile([NB, TAGS], F32, tag="s")
        nc.vector.tensor_add(s2[:], v[:], trs_sb[:])
        m2 = vp.tile([NB, 1], F32, tag="m2")
        nc.vector.reduce_max(m2[:], s2[:], axis=AX.X)
        eq2 = vp.tile([NB, TAGS], F32, tag="eq")
        nc.vector.tensor_tensor(out=eq2[:], in0=s2[:],
                                in1=m2[:].to_broadcast((NB, TAGS)), op=OP.is_equal)
        pr2 = vp.tile([NB, TAGS], F32, tag="prod")
        nc.vector.tensor_mul(pr2[:], eq2[:], iod[:])
        enc = vp.tile([NB, 1], F32, tag="enc")
        nc.vector.reduce_max(enc[:], pr2[:], axis=AX.X)
        tag = vpers.tile([NB, 1], F32, name="tag")
        nc.vector.tensor_copy(tag[:], enc[:])

        for t in range(SEQ - 1, -1, -1):
            nc.vector.tensor_copy(tags_sb[:, t:t + 1], tag[:])
            if t == 0:
                break
            pr = vp.tile([NB, TAGS], F32, tag="prod")
            nc.vector.scalar_tensor_tensor(
                out=pr[:], in0=iod[:], scalar=tag[:], in1=bp_sb[:, ts(t, TAGS)],
                op0=OP.is_equal, op1=OP.mult)
            nc.vector.reduce_max(tag[:], pr[:], axis=AX.X)

        nc.sync.dma_start(path[:, :], tags_sb[:])

    nc.compile()
    return nc


_NC_CACHE = {}


def _get_nc():
    if "nc" not in _NC_CACHE:
        _NC_CACHE["nc"] = build_nc()
    return _NC_CACHE["nc"]


def kernel(sentence, emb, lstm_params, fc_W, fc_b, transitions):
    sentence = np.asarray(sentence)
    emb = np.asarray(emb, np.float32)
    fc_W = np.asarray(fc_W, np.float32)
    fc_b = np.asarray(fc_b, np.float32)
    transitions = np.asarray(transitions, np.float32)
    perm = _perm_gates()

    common = {"emb": emb.astype(ml_dtypes.bfloat16)}
    for l, lp in enumerate(lstm_params):
        kin = EMB if l == 0 else 2 * HID
        kpad = EMBP if l == 0 else 2 * HID
        for di, d in enumerate(("f", "b")):
            Wih = np.asarray(lp[3 * di + 0], np.float32)[perm]      # [2048, kin]
            Whh = np.asarray(lp[3 * di + 1], np.float32)[perm]      # [2048, 512]
            b = np.asarray(lp[3 * di + 2], np.float32)[perm]        # [2048]
            WihT = np.zeros((kpad, G4), np.float32)
            WihT[:kin] = Wih.T
            common[f"wih{l}{d}"] = np.ascontiguousarray(
                WihT.reshape(kpad // 128, 128, G4).transpose(1, 0, 2)
            ).astype(ml_dtypes.bfloat16)
            common[f"whh{l}{d}"] = np.ascontiguousarray(
                Whh.T.reshape(KH, 128, G4).transpose(1, 0, 2)
            ).astype(ml_dtypes.float8_e4m3fn)
            common[f"b{l}{d}"] = np.ascontiguousarray(
                b.reshape(MCH, 128).T).astype(np.float32)
    common["fcw"] = np.ascontiguousarray(
        fc_W.T.reshape(8, 128, TAGS).transpose(1, 0, 2)).astype(ml_dtypes.bfloat16)

    transp = transitions + fc_b[:, None]   # fold fc bias into transition rows
    common["trans8"] = np.tile(transp.reshape(1, -1), (NB, 1)).astype(np.float32)
    common["trstop"] = np.tile(transitions[STOP].reshape(1, -1), (NB, 1)).astype(
        np.float32)
    v0 = np.full((TAGS,), -10000.0, np.float32)
    v0[START] = 0.0
    common["v0"] = np.tile(v0.reshape(1, -1), (NB, 1))
    common["iota20"] = np.tile(np.arange(TAGS, dtype=np.float32).reshape(1, -1),
                               (NB, 1))
    common["iodesc"] = np.tile(
        (TAGS - np.arange(TAGS, dtype=np.float32)).reshape(1, -1), (NB, 1))

    in_maps = []
    for i in range(NCORES):
        m = dict(common)
        toks = np.asarray(sentence[:, i * NB:(i + 1) * NB], np.int32).reshape(-1)
        m["tok"] = np.ascontiguousarray(toks.reshape(NT // 128, 128).T)
        in_maps.append(m)

    nc = _get_nc()
    trace = os.environ.get("KERNEL_TRACE", "0") == "1"
    res = run_bass_kernel_spmd(nc, in_maps, core_ids=list(range(NCORES)),
                               trace=trace)
    if trace:
        print(f"HW exec time: {res.exec_time_ns} ns")
    out = np.concatenate([r["path"] for r in res.results], axis=0)
    return (TAGS - out).astype(np.float32)


if __name__ == "__main__":
    sys.path.insert(0, "/root/problem")
    from reference import setup_inputs, reference
    inp = setup_inputs()
    inp = {k: np.asarray(v) if not isinstance(v, tuple) else v
           for k, v in inp.items()}
    got = kernel(**inp)
    print("kernel output", got.shape, got.dtype)
